# revision 38
# baseline (speedup 1.0000x reference)
"""Trainium2 Bass kernel for ConstantTimeStrideAttention.

Model (reference.py):
  qkv = x @ Wqkv + bqkv -> q,k,v per head (B=2, S=2048, DIM=1536, H=12, HD=128)
  per query s: 12 anchors (6 local +-1..3, 4 strided +-5,+-10, 2 global {0,S-1})
  attn = softmax(q . k_anchor * HD^-0.5 + log(group_weight)); out = attn @ v_anchors
  y = concat_heads @ Wout + bout
Sharding: 8 cores = (2 batches) x (4 sequence chunks of 512 queries). Each core
recomputes the k/v halo (+-10 tokens) and the two global tokens from the full x
input, so there are no collectives.

Device layout is feature-major ("transposed"): xT [DIM, 536 ext tokens] ->
qT/kT per head [128, toks]; V is produced token-major [toks, feats] so that the
attention AV matmul needs no transposes. Scores are computed transposed
(PT[k, q]) via kT-stationary matmuls; softmax runs as exp (ScalarE) ->
mask-multiply (VectorE; the mask carries the per-group softmax weights on the
anchor diagonals and zeroes everything else) -> denominator via an all-ones
stationary matmul that replicates the per-query sum across all 128 partitions
(so the reciprocal needs no partition broadcast) -> AV matmul -> multiply by
replicated reciprocal during PSUM evacuation. The per-token 1/denominator
commutes through the output projection, which consumes attn_T directly.
The V bias commutes through attention exactly (softmax weights sum to 1), so
it is folded into bout on the host: bout_eff = bout + bqkv_v @ Wout.

All tensors stream as bfloat16 (fp32 PSUM accumulation): halves HBM traffic vs
fp32 (weights dominate: ~19MB/core), enables FWL fast weight loads, and doubles
VectorE throughput on the softmax elementwise work. Weight/x slices are packed
in f-chunk pairs so each DMA moves 2KB per partition. DMA ring assignment and
wq_pool buffer rotation (3 bufs) throttle weight streaming to need-order so the
startup-critical xt + K-weight transfers get the shared ~330GB/s. Per block,
the K projection runs first (its PSUM pattern pipelines with the incoming DMA
slices), then V, then Q/attention with Q(h+1) emitted ahead of attention(h) so
the PE never idles while VectorE evacuates qT. A memset-fed warmup matmul burst
during the fixed ~8us engine-init window pre-warms the PE HAM clock-gate.
"""

import sys

sys.path.insert(0, "/opt/trn_rl_repo")

import ml_dtypes  # noqa: E402
import numpy as np  # noqa: E402

import concourse.bass as bass  # noqa: E402,F401
import concourse.tile as tile  # noqa: E402
from concourse import bacc, mybir  # noqa: E402
from concourse import bass_utils  # noqa: E402

F32 = mybir.dt.float32
BF16 = mybir.dt.bfloat16
NPBF16 = ml_dtypes.bfloat16
EXP = mybir.ActivationFunctionType.Exp
COPY = mybir.ActivationFunctionType.Copy
IDENT = mybir.ActivationFunctionType.Identity

B, S, DIM = 2, 2048, 1536
H, HD = 12, 128
SIGMA = 12
NCORES = 8
SCHUNKS = 4          # sequence chunks per batch
Q = S // SCHUNKS     # 512 queries per core
WIN = 10             # halo: max |offset|
EXT = 2 + (Q + 2 * WIN) + 2   # 536 ext k/v columns: [g0 g1][window 532][g0 g1]
NF = DIM // 128      # 12 contraction chunks
NJ = NF // 2         # 6 f-chunk pairs (DMA slice granularity)
OFFS = [-3, -2, -1, 1, 2, 3, -10, -5, 5, 10]
# k-row chunks per query tile (256 queries each): (start, size) in ext cols
CHUNKS = [[(0, 128), (128, 128), (256, 128)],
          [(256, 128), (384, 128), (512, EXT - 512)]]

_CACHE = {}


def _build_program():
    nc = bacc.Bacc("TRN2", target_bir_lowering=False, debug=False)

    # x ext, transposed + packed in f-pairs: [j, 128, 2*EXT]
    xt_d = nc.dram_tensor("xt", [NJ, 128, 2 * EXT], BF16, kind="ExternalInput").ap()
    # weights pre-tiled on host: [group, j, 128, 1024]; f-chunk f lives at
    # cols f*512 of the flattened [128, NF*512] SBUF tile
    wqkv_d = nc.dram_tensor("wqkv", [9, NJ, 128, 1024], BF16, kind="ExternalInput").ap()
    wout_d = nc.dram_tensor("wout", [3, NJ, 128, 1024], BF16, kind="ExternalInput").ap()
    # bias columns pre-transposed on host: [:, 0:24]=bq|bk per head, [:, 24:36]=bo
    bcol_d = nc.dram_tensor("bcol", [128, 3 * H], F32, kind="ExternalInput").ap()
    ones_d = nc.dram_tensor("ones_sq", [128, 128], BF16, kind="ExternalInput").ap()
    masks_d = nc.dram_tensor("masks", [6, 128, 256], BF16, kind="ExternalInput").ap()
    yt_d = nc.dram_tensor("yt", [DIM, Q], BF16, kind="ExternalOutput").ap()

    QCOL0 = 2 + WIN  # ext col of the first query token

    with tile.TileContext(nc) as tc:
        const = tc.alloc_tile_pool(name="const", bufs=1)
        at_pool = tc.alloc_tile_pool(name="at", bufs=1)
        xt_pool = tc.alloc_tile_pool(name="xt", bufs=1)
        wq_pool = tc.alloc_tile_pool(name="wq", bufs=3)
        qT_pool = tc.alloc_tile_pool(name="qT", bufs=6)
        kT_pool = tc.alloc_tile_pool(name="kT", bufs=6)
        v_pool = tc.alloc_tile_pool(name="v", bufs=10)
        et_pool = tc.alloc_tile_pool(name="et", bufs=3)
        ptm_pool = tc.alloc_tile_pool(name="ptm", bufs=3)
        rec_pool = tc.alloc_tile_pool(name="rec", bufs=2)

        # ---- PE warmup: memset-fed matmul burst so the HAM clock-gate is at
        # 8/8 before the first real matmul (data arrives ~10us in; engine init
        # runs ~7us; the burst fills the gap with garbage-in matmuls).
        warm_ps = tc.alloc_tile_pool(name="warm_ps", bufs=1, space="PSUM")
        warm_t = const.tile([128, 256], BF16, tag="warm")
        nc.vector.memset(warm_t[:], 0.0)
        wps = warm_ps.tile([128, 256], F32)
        for i in range(16):
            nc.tensor.matmul(
                wps[:], warm_t[:, 0:128], warm_t[:],
                start=(i == 0), stop=(i == 15),
            )
        warm_ps.release()

        qk_ps = tc.alloc_tile_pool(name="qk_ps", bufs=2, space="PSUM")
        v_ps = tc.alloc_tile_pool(name="v_ps", bufs=2, space="PSUM")
        pt_ps = tc.alloc_tile_pool(name="pt_ps", bufs=2, space="PSUM")
        av_ps = tc.alloc_tile_pool(name="av_ps", bufs=1, space="PSUM")
        dn_ps = tc.alloc_tile_pool(name="dn_ps", bufs=1, space="PSUM")

        # bias columns first on the scalar ring (tiny; unblocks kT evacuation)
        bcol_t = const.tile([128, 3 * H], F32, tag="bcol")
        nc.scalar.dma_start(bcol_t[:], bcol_d[:])
        bqk_t = [bcol_t[:, i : i + 1] for i in range(2 * H)]
        bo_t = [bcol_t[:, 2 * H + i : 2 * H + i + 1] for i in range(H)]

        # ---- startup-critical transfers, striped round-robin across all 3
        # rings in need order so per-queue bandwidth sharing serves the
        # critical path first: xt/g3 pairs interleaved (K consumes them in f
        # order), then g6 (V), then g0 (Q) + ones/masks.
        RINGS = [nc.sync, nc.scalar, nc.gpsimd]
        _stripe = [0]

        def ring():
            r = RINGS[_stripe[0] % len(RINGS)]
            _stripe[0] += 1
            return r

        PJW = 2 * EXT  # 1072 cols per pair slice
        xt_t = xt_pool.tile([128, NJ * PJW], BF16, tag="xt")
        wq_tiles = {}

        def wq_tile():
            t = wq_pool.tile([128, NF * 512], BF16, tag="wqg")
            return t

        for g in (3, 6, 0):
            wq_tiles[g] = wq_tile()

        def _crit_dma(g, j, e=None):
            # one pair slice (or f-granular half) of weight group g
            if e is None:
                ring().dma_start(
                    wq_tiles[g][:, j * 1024 : (j + 1) * 1024], wqkv_d[g, j]
                )
            else:
                ring().dma_start(
                    wq_tiles[g][:, j * 1024 + e * 512 : j * 1024 + (e + 1) * 512],
                    wqkv_d[g, j][:, e * 512 : (e + 1) * 512],
                )

        # xt/g3/g6 fully f-granular and interleaved in f order: the ramp
        # f-major K+V loop consumes each (xt_f, g3_f, g6_f) triple as it
        # lands, and small transfers complete sooner under the per-ring
        # 2-transfer interleaving. g0 (Q) follows in pairs.
        for fi in range(NF):
            j, e = fi // 2, fi % 2
            ring().dma_start(
                xt_t[:, j * PJW + e * EXT : j * PJW + (e + 1) * EXT],
                xt_d[j][:, e * EXT : (e + 1) * EXT],
            )
            _crit_dma(3, j, e)
            _crit_dma(6, j, e)
        # g0 (Q) pairs with the tiny ones/mask tiles interleaved so the
        # first attention head is not gated behind the full g0 group
        ones_t = const.tile([128, 128], BF16, tag="ones")
        mask_t = [
            const.tile([128, 256], BF16, name="mask", tag=f"mask{i}")
            for i in range(6)
        ]
        ring().dma_start(ones_t[:], ones_d[:])
        for j in range(NJ):
            _crit_dma(0, j)
            ring().dma_start(mask_t[j][:], masks_d[j])
        xt = [
            xt_t[:, (f // 2) * PJW + (f % 2) * EXT :][:, :EXT] for f in range(NF)
        ]
        wt_g = {
            g: [wq_tiles[g][:, f * 512 : (f + 1) * 512] for f in range(NF)]
            for g in (3, 6, 0)
        }

        def load_wq(g, eng, src_d=wqkv_d):
            # 6 pair-slice DMAs (256KB each): consuming matmuls unblock as
            # slices land; 2KB-per-partition packets keep the DMA efficient.
            t = wq_pool.tile([128, NF * 512], BF16, tag="wqg")
            for j in range(NJ):
                eng.dma_start(t[:, j * 1024 : (j + 1) * 1024], src_d[g, j])
            return [t[:, f * 512 : (f + 1) * 512] for f in range(NF)]

        # later groups in need-order (K, V, Q per block) on the gpsimd ring;
        # wq_pool's 3 buffers flow-control the stream to stay near need time
        wt_g[4] = load_wq(4, nc.gpsimd)
        wt_g[7] = load_wq(7, nc.gpsimd)
        wt_g[1] = load_wq(1, nc.gpsimd)
        wt_g[5] = load_wq(5, nc.gpsimd)
        wt_g[8] = load_wq(8, nc.gpsimd)
        wt_g[2] = load_wq(2, nc.gpsimd)
        wt_o = [load_wq(og, nc.gpsimd, src_d=wout_d) for og in range(3)]

        # token chunks of the ext axis (for token-major V)
        TCH = [(c * 128, min(128, EXT - c * 128)) for c in range((EXT + 127) // 128)]

        qT = [None] * H
        kT = [None] * H
        vv = [[None] * 3 for _ in TCH]
        at = [None] * H

        def emit_v_group(g, wt, pairs=((0, 1), (2, 3), (4,))):
            # chunk pairs share the two v_ps banks with an interleaved f-loop
            # so the matmuls pace with incoming weight slices (no PE stall
            # waiting for the whole group during the DMA ramp)
            for pr in pairs:
                ps = {c: v_ps.tile([TCH[c][1], 512], F32, name="v_ps", tag="v_ps") for c in pr}
                for f in range(NF):
                    for c in pr:
                        cs, csz = TCH[c]
                        nc.tensor.matmul(
                            ps[c][:], xt[f][:, cs : cs + csz], wt[f][:],
                            start=(f == 0), stop=(f == NF - 1),
                        )
                for c in pr:
                    sb = v_pool.tile([TCH[c][1], 512], BF16, tag="v")
                    nc.scalar.activation(sb[:], ps[c][:], COPY)
                    vv[c][g] = sb

        def emit_q_chunk(hcur, wcol):
            hx = hcur % 4
            ps = qk_ps.tile([128, Q], F32)
            for f in range(NF):
                nc.tensor.matmul(
                    ps[:], wcol(f, hx),
                    xt[f][:, QCOL0 : QCOL0 + Q],
                    start=(f == 0), stop=(f == NF - 1),
                )
            sb = qT_pool.tile([128, Q], BF16, tag="qT")
            nc.scalar.activation(sb[:], ps[:], IDENT, bias=bqk_t[hcur][:])
            qT[hcur] = sb

        def emit_k_block(blk, wt, hx0=0):
            half = EXT // 2
            for hx in range(hx0, 4):
                hcur = blk * 4 + hx
                sb = kT_pool.tile([128, EXT], BF16, tag="kT")
                for j in range(2):
                    ps = qk_ps.tile([128, half], F32)
                    for f in range(NF):
                        nc.tensor.matmul(
                            ps[:], wt[f][:, hx * 128 : (hx + 1) * 128],
                            xt[f][:, j * half : (j + 1) * half],
                            start=(f == 0), stop=(f == NF - 1),
                        )
                    nc.vector.tensor_scalar_add(
                        sb[:, j * half : (j + 1) * half], ps[:], bqk_t[H + hcur][:]
                    )
                kT[hcur] = sb

        def emit_kv_ramp(wtk, wtv):
            # blk0 ramp: f-major joint loop over K heads hx0-2 (both halves)
            # and V chunks (0,1), borrowing the idle attention PSUM banks
            # (pt/av/dn) for K so the PE consumes each (xt,g3,g6) f-slice the
            # moment it lands. ~1.1us of matmuls per ~1.2us of delivery:
            # the whole ramp window is PE-fed.
            half = EXT // 2
            kslot = {
                (0, 0): qk_ps.tile([128, half], F32, name="ps"),
                (0, 1): qk_ps.tile([128, half], F32, name="ps"),
                (1, 0): pt_ps.tile([128, half], F32, name="ptp"),
                (1, 1): pt_ps.tile([128, half], F32, name="ptp"),
                (2, 0): av_ps.tile([128, half], F32, name="avp"),
                (2, 1): dn_ps.tile([128, half], F32, name="dnp"),
            }
            vps = {
                c: v_ps.tile([TCH[c][1], 512], F32, name="v_ps", tag="v_ps")
                for c in (0, 1)
            }
            for f in range(NF):
                for hx in range(3):
                    for j in range(2):
                        nc.tensor.matmul(
                            kslot[(hx, j)][:],
                            wtk[f][:, hx * 128 : (hx + 1) * 128],
                            xt[f][:, j * half : (j + 1) * half],
                            start=(f == 0), stop=(f == NF - 1),
                        )
                for c in (0, 1):
                    cs, csz = TCH[c]
                    nc.tensor.matmul(
                        vps[c][:], xt[f][:, cs : cs + csz], wtv[f][:],
                        start=(f == 0), stop=(f == NF - 1),
                    )
            for hx in range(3):
                sb = kT_pool.tile([128, EXT], BF16, tag="kT")
                for j in range(2):
                    nc.vector.tensor_scalar_add(
                        sb[:, j * half : (j + 1) * half], kslot[(hx, j)][:],
                        bqk_t[H + hx][:],
                    )
                kT[hx] = sb
            for c in (0, 1):
                vsb = v_pool.tile([TCH[c][1], 512], BF16, tag="v")
                nc.scalar.activation(vsb[:], vps[c][:], COPY)
                vv[c][0] = vsb

        def emit_attention(h, fill_steps=None):
            si = 0
            sb = at_pool.tile([128, Q], BF16, tag=f"at{h}")
            for t in range(2):
                avp = av_ps.tile([128, 256], F32)
                dnp = dn_ps.tile([128, 256], F32)
                nch = len(CHUNKS[t])
                for ci, (cs, csz) in enumerate(CHUNKS[t]):
                    ptp = pt_ps.tile([csz, 256], F32)
                    nc.tensor.matmul(
                        ptp[:], kT[h][:, cs : cs + csz],
                        qT[h][:, t * 256 : (t + 1) * 256],
                        start=True, stop=True,
                    )
                    et = et_pool.tile([csz, 256], BF16, tag="et")
                    nc.scalar.activation(et[:], ptp[:], EXP)
                    ptm = ptm_pool.tile([csz, 256], BF16, tag="ptm")
                    nc.vector.tensor_mul(ptm[:], et[:], mask_t[t * 3 + ci][:csz, :])
                    # fill between PT and AV: the fill matmuls run while
                    # ScalarE/VectorE produce ptm, absorbing AV's wait
                    if fill_steps and si < len(fill_steps):
                        fill_steps[si]()
                        si += 1
                    nc.tensor.matmul(
                        avp[:], vv[cs // 128][h // 4][:csz, (h % 4) * 128 : (h % 4 + 1) * 128],
                        ptm[:], start=(ci == 0), stop=(ci == nch - 1),
                    )
                    nc.tensor.matmul(
                        dnp[:], ones_t[:csz, :], ptm[:],
                        start=(ci == 0), stop=(ci == nch - 1),
                    )
                rec = rec_pool.tile([128, 256], F32, tag="rec")
                nc.vector.reciprocal_approx_fast(rec[:], dnp[:])
                nc.vector.tensor_mul(sb[:, t * 256 : (t + 1) * 256], avp[:], rec[:])
            at[h] = sb

        def q_fill_steps(hcur, wq_col):
            # Q(h+1) as six 2-matmul steps interleaved into attention(h)'s
            # chunks, filling the exp->mask-multiply pipeline stalls
            hx = hcur % 4
            ps = qk_ps.tile([128, Q], F32, name="ps")
            steps = []
            for fs in (range(0, 2), range(2, 4), range(4, 6),
                       range(6, 8), range(8, 10), range(10, 12)):
                def go(fs=fs, hcur=hcur, hx=hx):
                    for f in fs:
                        nc.tensor.matmul(
                            ps[:], wq_col(f, hx), xt[f][:, QCOL0 : QCOL0 + Q],
                            start=(f == 0), stop=(f == NF - 1),
                        )
                    if fs.stop == NF:
                        sb = qT_pool.tile([128, Q], BF16, tag="qT")
                        nc.scalar.activation(
                            sb[:], ps[:], IDENT, bias=bqk_t[hcur][:]
                        )
                        qT[hcur] = sb
                steps.append(go)
            return steps

        def k_hx0_fill(blkn, wt):
            # next block's first K head, split into 6 four-matmul steps that
            # fill the last attention head's pipeline stalls (PE is strict
            # FIFO: fill work must be emitted between the stalling matmuls)
            half = EXT // 2
            kfp = [qk_ps.tile([128, half], F32, name="ps") for _ in range(2)]
            sb = kT_pool.tile([128, EXT], BF16, tag="kT")
            steps = []
            for j in range(2):
                for fs in (range(0, 4), range(4, 8), range(8, 12)):
                    def go(j=j, fs=fs):
                        for f in fs:
                            nc.tensor.matmul(
                                kfp[j][:], wt[f][:, 0:128],
                                xt[f][:, j * half : (j + 1) * half],
                                start=(f == 0), stop=(f == NF - 1),
                            )
                        if fs.stop == NF:
                            nc.vector.tensor_scalar_add(
                                sb[:, j * half : (j + 1) * half], kfp[j][:],
                                bqk_t[H + blkn * 4][:],
                            )
                    steps.append(go)
            kT[blkn * 4] = sb
            return steps

        # ---- emission order per block: K (pipelines with incoming DMA
        # slices), V, then Q/attention with Q(h+1) ahead of attention(h).
        # blk0 starts with the joint K+V ramp loop.
        for blk in range(3):
            h0 = blk * 4
            if blk == 0:
                emit_kv_ramp(wt_g[3], wt_g[6])
                emit_k_block(blk, wt_g[3], hx0=3)
                emit_v_group(blk, wt_g[6], pairs=((2, 3), (4,)))
            else:
                emit_k_block(blk, wt_g[3 + blk], hx0=1)
                emit_v_group(blk, wt_g[6 + blk])
            wtb = wt_g[blk]
            wq_col = lambda f, hx, _w=wtb: _w[f][:, hx * 128 : (hx + 1) * 128]
            emit_q_chunk(h0, wq_col)
            for h in range(h0, h0 + 4):
                if h + 1 < h0 + 4:
                    fill = q_fill_steps(h + 1, wq_col)
                elif blk < 2:
                    fill = k_hx0_fill(blk + 1, wt_g[4 + blk])
                else:
                    fill = None
                emit_attention(h, fill)

        # release stage-1 PSUM pools (reverse alloc order) before the projection
        for p in (dn_ps, av_ps, pt_ps, v_ps, qk_ps):
            p.release()

        yt_sb_pool = tc.alloc_tile_pool(name="yt_sb", bufs=2)
        yt_ps = tc.alloc_tile_pool(name="yt_ps", bufs=2, space="PSUM")

        for og in range(3):
            wt = wt_o[og]
            for oc in range(4):
                o = og * 4 + oc
                if o < H - 1:
                    ps = yt_ps.tile([128, Q], F32)
                    for f in range(NF):
                        nc.tensor.matmul(
                            ps[:], wt[f][:, oc * 128 : (oc + 1) * 128], at[f][:],
                            start=(f == 0), stop=(f == NF - 1),
                        )
                    sb = yt_sb_pool.tile([128, Q], BF16, tag="yt")
                    nc.vector.tensor_scalar_add(sb[:], ps[:], bo_t[o][:])
                    nc.sync.dma_start(yt_d[o * 128 : (o + 1) * 128, :], sb[:])
                else:
                    # final chunk: two independent half-width chains (separate
                    # PSUM tiles, so no WAR serialization) — the first half's
                    # evac + store overlap the second half's matmul tail
                    for hf in range(2):
                        cols = slice(hf * 256, (hf + 1) * 256)
                        ps = yt_ps.tile([128, 256], F32, name="ps_h", tag="yt_h")
                        for f in range(NF):
                            nc.tensor.matmul(
                                ps[:], wt[f][:, oc * 128 : (oc + 1) * 128],
                                at[f][:, cols],
                                start=(f == 0), stop=(f == NF - 1),
                            )
                        sb = yt_sb_pool.tile([128, 256], BF16, tag="yt_h")
                        nc.vector.tensor_scalar_add(sb[:], ps[:], bo_t[o][:])
                        nc.sync.dma_start(
                            yt_d[o * 128 : (o + 1) * 128, cols], sb[:]
                        )

        yt_ps.release()
        for p in (yt_sb_pool, rec_pool, ptm_pool, et_pool, v_pool, kT_pool,
                  qT_pool, wq_pool, xt_pool, at_pool, const):
            p.release()

    nc.compile()
    return nc


def _softmax(v):
    e = np.exp(v - v.max())
    return e / e.sum()


def _build_masks(r0, gw):
    """Per-core mask tiles [6, 128, 256] routing softmax group weights onto the
    anchor positions of the transposed score chunks."""
    lo = r0 - WIN
    masks = np.zeros((6, 128, 256), np.float32)
    wts = [gw[0]] * 6 + [gw[1]] * 4
    for qi in range(Q):
        t, qq = divmod(qi, 256)

        def add(col, w):
            for ci, (cs, csz) in enumerate(CHUNKS[t]):
                if cs <= col < cs + csz:
                    masks[t * 3 + ci, col - cs, qq] += w
                    return
            raise AssertionError(f"col {col} not covered for qtile {t}")

        for off, w in zip(OFFS, wts):
            tok = min(max(r0 + qi + off, 0), S - 1)
            add(2 + (tok - lo), w)
        # global anchors: duplicated at both ends of the ext axis
        add(0 if t == 0 else EXT - 2, gw[2])   # token 0
        add(1 if t == 0 else EXT - 1, gw[2])   # token S-1
    return masks


def _prepare_in_maps(x, wqkv, bqkv, wout, bout, group_scale):
    scale = HD ** -0.5
    wqkv_m = np.array(wqkv, np.float32, copy=True)
    wqkv_m[:, :DIM] *= scale
    # pre-tile: [9 groups, 6 f-pairs, 128, 1024]; f-chunk f of group g lands
    # at flattened cols f*512 of the [128, NF*512] SBUF tile
    wqkv_t = np.ascontiguousarray(
        wqkv_m.reshape(NJ, 2, 128, 9, 512).transpose(3, 0, 2, 1, 4)
        .reshape(9, NJ, 128, 1024)
    ).astype(NPBF16)
    bqkv_m = np.array(bqkv, np.float32, copy=True)
    bqkv_m[:DIM] *= scale
    gw = _softmax(np.asarray(group_scale, np.float64))

    # V bias commutes through attention (softmax weights sum to 1): fold into
    # the output-projection bias
    wout_f = np.asarray(wout, np.float32)
    bout_eff = np.asarray(bout, np.float32) + bqkv_m[2 * DIM :] @ wout_f

    # bias columns [128, 36]: q heads, k heads, then out-proj chunks
    bcol = np.concatenate(
        [
            bqkv_m[:DIM].reshape(H, 128),
            bqkv_m[DIM : 2 * DIM].reshape(H, 128),
            bout_eff.reshape(H, 128),
        ],
        axis=0,
    ).T.astype(np.float32).copy()  # [128, 36]
    wout_t = np.ascontiguousarray(
        wout_f.reshape(NJ, 2, 128, 3, 512)
        .transpose(3, 0, 2, 1, 4).reshape(3, NJ, 128, 1024)
    ).astype(NPBF16)
    ones_sq = np.ones((128, 128), NPBF16)

    in_maps = []
    for core in range(NCORES):
        b, sc = divmod(core, SCHUNKS)
        r0 = sc * Q
        lo = r0 - WIN
        tok_ids = np.concatenate(
            [
                [0, S - 1],
                np.clip(np.arange(lo, lo + Q + 2 * WIN), 0, S - 1),
                [0, S - 1],
            ]
        ).astype(np.int64)
        x_ext_t = np.ascontiguousarray(x[b, tok_ids, :].T)  # [DIM, EXT]
        # pack f-chunk pairs: [NJ, 128, 2*EXT]
        xt_p = np.ascontiguousarray(
            x_ext_t.reshape(NJ, 2, 128, EXT).transpose(0, 2, 1, 3)
            .reshape(NJ, 128, 2 * EXT)
        ).astype(NPBF16)
        masks = _build_masks(r0, gw).astype(NPBF16)
        in_maps.append(
            {
                "xt": xt_p,
                "wqkv": wqkv_t,
                "wout": wout_t,
                "bcol": bcol,
                "ones_sq": ones_sq,
                "masks": masks,
            }
        )
    return in_maps


def get_program():
    if "nc" not in _CACHE:
        _CACHE["nc"] = _build_program()
    return _CACHE["nc"]


def run(inputs, **spmd_kwargs):
    """Run the SPMD kernel; returns (y [B,S,DIM] fp32, BassKernelResults)."""
    x = np.asarray(inputs["x"], np.float32)
    in_maps = _prepare_in_maps(
        x,
        np.asarray(inputs["Wqkv"], np.float32),
        np.asarray(inputs["bqkv"], np.float32),
        np.asarray(inputs["Wout"], np.float32),
        np.asarray(inputs["bout"], np.float32),
        np.asarray(inputs["group_scale"], np.float32),
    )
    nc = get_program()
    res = bass_utils.run_bass_kernel_spmd(
        nc, in_maps, core_ids=list(range(NCORES)), **spmd_kwargs
    )
    y = np.empty((B, S, DIM), np.float32)
    for core in range(NCORES):
        b, sc = divmod(core, SCHUNKS)
        y[b, sc * Q : (sc + 1) * Q, :] = res.results[core]["yt"].astype(np.float32).T
    return y, res


def kernel(**inputs):
    y, _ = run(inputs)
    return y


# revision 39
# speedup vs baseline: 1.0051x; 1.0051x over previous
"""Trainium2 Bass kernel for ConstantTimeStrideAttention.

Model (reference.py):
  qkv = x @ Wqkv + bqkv -> q,k,v per head (B=2, S=2048, DIM=1536, H=12, HD=128)
  per query s: 12 anchors (6 local +-1..3, 4 strided +-5,+-10, 2 global {0,S-1})
  attn = softmax(q . k_anchor * HD^-0.5 + log(group_weight)); out = attn @ v_anchors
  y = concat_heads @ Wout + bout
Sharding: 8 cores = (2 batches) x (4 sequence chunks of 512 queries). Each core
recomputes the k/v halo (+-10 tokens) and the two global tokens from the full x
input, so there are no collectives.

Device layout is feature-major ("transposed"): xT [DIM, 536 ext tokens] ->
qT/kT per head [128, toks]; V is produced token-major [toks, feats] so that the
attention AV matmul needs no transposes. Scores are computed transposed
(PT[k, q]) via kT-stationary matmuls; softmax runs as exp (ScalarE) ->
mask-multiply (VectorE; the mask carries the per-group softmax weights on the
anchor diagonals and zeroes everything else) -> denominator via an all-ones
stationary matmul that replicates the per-query sum across all 128 partitions
(so the reciprocal needs no partition broadcast) -> AV matmul -> multiply by
replicated reciprocal during PSUM evacuation. The per-token 1/denominator
commutes through the output projection, which consumes attn_T directly.
The V bias commutes through attention exactly (softmax weights sum to 1), so
it is folded into bout on the host: bout_eff = bout + bqkv_v @ Wout.

All tensors stream as bfloat16 (fp32 PSUM accumulation): halves HBM traffic vs
fp32 (weights dominate: ~19MB/core), enables FWL fast weight loads, and doubles
VectorE throughput on the softmax elementwise work. Weight/x slices are packed
in f-chunk pairs so each DMA moves 2KB per partition. DMA ring assignment and
wq_pool buffer rotation (3 bufs) throttle weight streaming to need-order so the
startup-critical xt + K-weight transfers get the shared ~330GB/s. Per block,
the K projection runs first (its PSUM pattern pipelines with the incoming DMA
slices), then V, then Q/attention with Q(h+1) emitted ahead of attention(h) so
the PE never idles while VectorE evacuates qT. A memset-fed warmup matmul burst
during the fixed ~8us engine-init window pre-warms the PE HAM clock-gate.
"""

import sys

sys.path.insert(0, "/opt/trn_rl_repo")

import ml_dtypes  # noqa: E402
import numpy as np  # noqa: E402

import concourse.bass as bass  # noqa: E402,F401
import concourse.tile as tile  # noqa: E402
from concourse import bacc, mybir  # noqa: E402
from concourse import bass_utils  # noqa: E402

F32 = mybir.dt.float32
BF16 = mybir.dt.bfloat16
NPBF16 = ml_dtypes.bfloat16
EXP = mybir.ActivationFunctionType.Exp
COPY = mybir.ActivationFunctionType.Copy
IDENT = mybir.ActivationFunctionType.Identity

B, S, DIM = 2, 2048, 1536
H, HD = 12, 128
SIGMA = 12
NCORES = 8
SCHUNKS = 4          # sequence chunks per batch
Q = S // SCHUNKS     # 512 queries per core
WIN = 10             # halo: max |offset|
EXT = 2 + (Q + 2 * WIN) + 2   # 536 ext k/v columns: [g0 g1][window 532][g0 g1]
NF = DIM // 128      # 12 contraction chunks
NJ = NF // 2         # 6 f-chunk pairs (DMA slice granularity)
OFFS = [-3, -2, -1, 1, 2, 3, -10, -5, 5, 10]
# k-row chunks per query tile (256 queries each): (start, size) in ext cols
CHUNKS = [[(0, 128), (128, 128), (256, 128)],
          [(256, 128), (384, 128), (512, EXT - 512)]]

_CACHE = {}


def _build_program():
    nc = bacc.Bacc("TRN2", target_bir_lowering=False, debug=False)

    # x ext, transposed + packed in f-pairs: [j, 128, 2*EXT]
    xt_d = nc.dram_tensor("xt", [NJ, 128, 2 * EXT], BF16, kind="ExternalInput").ap()
    # weights pre-tiled on host: [group, j, 128, 1024]; f-chunk f lives at
    # cols f*512 of the flattened [128, NF*512] SBUF tile
    wqkv_d = nc.dram_tensor("wqkv", [9, NJ, 128, 1024], BF16, kind="ExternalInput").ap()
    wout_d = nc.dram_tensor("wout", [3, NJ, 128, 1024], BF16, kind="ExternalInput").ap()
    # bias columns pre-transposed on host: [:, 0:24]=bq|bk per head, [:, 24:36]=bo
    bcol_d = nc.dram_tensor("bcol", [128, 3 * H], F32, kind="ExternalInput").ap()
    ones_d = nc.dram_tensor("ones_sq", [128, 128], BF16, kind="ExternalInput").ap()
    masks_d = nc.dram_tensor("masks", [6, 128, 256], BF16, kind="ExternalInput").ap()
    yt_d = nc.dram_tensor("yt", [DIM, Q], BF16, kind="ExternalOutput").ap()

    QCOL0 = 2 + WIN  # ext col of the first query token

    with tile.TileContext(nc) as tc:
        const = tc.alloc_tile_pool(name="const", bufs=1)
        at_pool = tc.alloc_tile_pool(name="at", bufs=1)
        xt_pool = tc.alloc_tile_pool(name="xt", bufs=1)
        wq_pool = tc.alloc_tile_pool(name="wq", bufs=3)
        qT_pool = tc.alloc_tile_pool(name="qT", bufs=6)
        kT_pool = tc.alloc_tile_pool(name="kT", bufs=6)
        v_pool = tc.alloc_tile_pool(name="v", bufs=10)
        et_pool = tc.alloc_tile_pool(name="et", bufs=3)
        ptm_pool = tc.alloc_tile_pool(name="ptm", bufs=3)
        rec_pool = tc.alloc_tile_pool(name="rec", bufs=2)

        # ---- PE warmup: memset-fed matmul burst so the HAM clock-gate is at
        # 8/8 before the first real matmul (data arrives ~10us in; engine init
        # runs ~7us; the burst fills the gap with garbage-in matmuls).
        warm_ps = tc.alloc_tile_pool(name="warm_ps", bufs=1, space="PSUM")
        warm_t = const.tile([128, 256], BF16, tag="warm")
        nc.vector.memset(warm_t[:], 0.0)
        wps = warm_ps.tile([128, 256], F32)
        for i in range(16):
            nc.tensor.matmul(
                wps[:], warm_t[:, 0:128], warm_t[:],
                start=(i == 0), stop=(i == 15),
            )
        warm_ps.release()

        qk_ps = tc.alloc_tile_pool(name="qk_ps", bufs=2, space="PSUM")
        v_ps = tc.alloc_tile_pool(name="v_ps", bufs=2, space="PSUM")
        pt_ps = tc.alloc_tile_pool(name="pt_ps", bufs=2, space="PSUM")
        av_ps = tc.alloc_tile_pool(name="av_ps", bufs=1, space="PSUM")
        dn_ps = tc.alloc_tile_pool(name="dn_ps", bufs=1, space="PSUM")

        # bias columns first on the scalar ring (tiny; unblocks kT evacuation)
        bcol_t = const.tile([128, 3 * H], F32, tag="bcol")
        nc.scalar.dma_start(bcol_t[:], bcol_d[:])
        bqk_t = [bcol_t[:, i : i + 1] for i in range(2 * H)]
        bo_t = [bcol_t[:, 2 * H + i : 2 * H + i + 1] for i in range(H)]

        # ---- startup-critical transfers, striped round-robin across all 3
        # rings in need order so per-queue bandwidth sharing serves the
        # critical path first: xt/g3 pairs interleaved (K consumes them in f
        # order), then g6 (V), then g0 (Q) + ones/masks.
        RINGS = [nc.sync, nc.scalar, nc.gpsimd]
        _stripe = [0]

        def ring():
            r = RINGS[_stripe[0] % len(RINGS)]
            _stripe[0] += 1
            return r

        PJW = 2 * EXT  # 1072 cols per pair slice
        xt_t = xt_pool.tile([128, NJ * PJW], BF16, tag="xt")
        wq_tiles = {}

        def wq_tile():
            t = wq_pool.tile([128, NF * 512], BF16, tag="wqg")
            return t

        for g in (3, 6, 0):
            wq_tiles[g] = wq_tile()

        def _crit_dma(g, j, e=None):
            # one pair slice (or f-granular half) of weight group g
            if e is None:
                ring().dma_start(
                    wq_tiles[g][:, j * 1024 : (j + 1) * 1024], wqkv_d[g, j]
                )
            else:
                ring().dma_start(
                    wq_tiles[g][:, j * 1024 + e * 512 : j * 1024 + (e + 1) * 512],
                    wqkv_d[g, j][:, e * 512 : (e + 1) * 512],
                )

        # xt/g3/g6 fully f-granular and interleaved in f order: the ramp
        # f-major K+V loop consumes each (xt_f, g3_f, g6_f) triple as it
        # lands, and small transfers complete sooner under the per-ring
        # 2-transfer interleaving. g0 (Q) follows in pairs.
        for fi in range(NF):
            j, e = fi // 2, fi % 2
            ring().dma_start(
                xt_t[:, j * PJW + e * EXT : j * PJW + (e + 1) * EXT],
                xt_d[j][:, e * EXT : (e + 1) * EXT],
            )
            _crit_dma(3, j, e)
            _crit_dma(6, j, e)
        # g0 (Q) pairs with the tiny ones/mask tiles interleaved so the
        # first attention head is not gated behind the full g0 group
        ones_t = const.tile([128, 128], BF16, tag="ones")
        mask_t = [
            const.tile([128, 256], BF16, name="mask", tag=f"mask{i}")
            for i in range(6)
        ]
        ring().dma_start(ones_t[:], ones_d[:])
        for j in range(NJ):
            _crit_dma(0, j)
            ring().dma_start(mask_t[j][:], masks_d[j])
        xt = [
            xt_t[:, (f // 2) * PJW + (f % 2) * EXT :][:, :EXT] for f in range(NF)
        ]
        wt_g = {
            g: [wq_tiles[g][:, f * 512 : (f + 1) * 512] for f in range(NF)]
            for g in (3, 6, 0)
        }

        def load_wq(g, eng, src_d=wqkv_d):
            # 6 pair-slice DMAs (256KB each): consuming matmuls unblock as
            # slices land; 2KB-per-partition packets keep the DMA efficient.
            t = wq_pool.tile([128, NF * 512], BF16, tag="wqg")
            for j in range(NJ):
                eng.dma_start(t[:, j * 1024 : (j + 1) * 1024], src_d[g, j])
            return [t[:, f * 512 : (f + 1) * 512] for f in range(NF)]

        # later groups in need-order (K, V, Q per block) on the gpsimd ring;
        # wq_pool's 3 buffers flow-control the stream to stay near need time
        wt_g[4] = load_wq(4, nc.gpsimd)
        wt_g[7] = load_wq(7, nc.gpsimd)
        wt_g[1] = load_wq(1, nc.gpsimd)
        wt_g[5] = load_wq(5, nc.gpsimd)
        wt_g[8] = load_wq(8, nc.gpsimd)
        wt_g[2] = load_wq(2, nc.gpsimd)
        wt_o = [load_wq(og, nc.gpsimd, src_d=wout_d) for og in range(3)]

        # token chunks of the ext axis (for token-major V)
        TCH = [(c * 128, min(128, EXT - c * 128)) for c in range((EXT + 127) // 128)]

        qT = [None] * H
        kT = [None] * H
        vv = [[None] * 3 for _ in TCH]
        at = [None] * H

        def emit_v_group(g, wt, pairs=((0, 1), (2, 3), (4,))):
            # chunk pairs share the two v_ps banks with an interleaved f-loop
            # so the matmuls pace with incoming weight slices (no PE stall
            # waiting for the whole group during the DMA ramp)
            for pr in pairs:
                ps = {c: v_ps.tile([TCH[c][1], 512], F32, name="v_ps", tag="v_ps") for c in pr}
                for f in range(NF):
                    for c in pr:
                        cs, csz = TCH[c]
                        nc.tensor.matmul(
                            ps[c][:], xt[f][:, cs : cs + csz], wt[f][:],
                            start=(f == 0), stop=(f == NF - 1),
                        )
                for c in pr:
                    sb = v_pool.tile([TCH[c][1], 512], BF16, tag="v")
                    nc.scalar.activation(sb[:], ps[c][:], COPY)
                    vv[c][g] = sb

        def emit_q_chunk(hcur, wcol):
            hx = hcur % 4
            ps = qk_ps.tile([128, Q], F32)
            for f in range(NF):
                nc.tensor.matmul(
                    ps[:], wcol(f, hx),
                    xt[f][:, QCOL0 : QCOL0 + Q],
                    start=(f == 0), stop=(f == NF - 1),
                )
            sb = qT_pool.tile([128, Q], BF16, tag="qT")
            nc.scalar.activation(sb[:], ps[:], IDENT, bias=bqk_t[hcur][:])
            qT[hcur] = sb

        def emit_k_block(blk, wt, hx0=0):
            half = EXT // 2
            for hx in range(hx0, 4):
                hcur = blk * 4 + hx
                sb = kT_pool.tile([128, EXT], BF16, tag="kT")
                for j in range(2):
                    ps = qk_ps.tile([128, half], F32)
                    for f in range(NF):
                        nc.tensor.matmul(
                            ps[:], wt[f][:, hx * 128 : (hx + 1) * 128],
                            xt[f][:, j * half : (j + 1) * half],
                            start=(f == 0), stop=(f == NF - 1),
                        )
                    nc.vector.tensor_scalar_add(
                        sb[:, j * half : (j + 1) * half], ps[:], bqk_t[H + hcur][:]
                    )
                kT[hcur] = sb

        def emit_kv_ramp(wtk, wtv):
            # blk0 ramp: f-major joint loop over K heads hx0-2 (both halves)
            # and V chunks (0,1), borrowing the idle attention PSUM banks
            # (pt/av/dn) for K so the PE consumes each (xt,g3,g6) f-slice the
            # moment it lands. ~1.1us of matmuls per ~1.2us of delivery:
            # the whole ramp window is PE-fed.
            half = EXT // 2
            kslot = {
                (0, 0): qk_ps.tile([128, half], F32, name="ps"),
                (0, 1): qk_ps.tile([128, half], F32, name="ps"),
                (1, 0): pt_ps.tile([128, half], F32, name="ptp"),
                (1, 1): pt_ps.tile([128, half], F32, name="ptp"),
                (2, 0): av_ps.tile([128, half], F32, name="avp"),
                (2, 1): dn_ps.tile([128, half], F32, name="dnp"),
            }
            vps = {
                c: v_ps.tile([TCH[c][1], 512], F32, name="v_ps", tag="v_ps")
                for c in (0, 1)
            }
            for f in range(NF):
                for hx in range(3):
                    for j in range(2):
                        nc.tensor.matmul(
                            kslot[(hx, j)][:],
                            wtk[f][:, hx * 128 : (hx + 1) * 128],
                            xt[f][:, j * half : (j + 1) * half],
                            start=(f == 0), stop=(f == NF - 1),
                        )
                for c in (0, 1):
                    cs, csz = TCH[c]
                    nc.tensor.matmul(
                        vps[c][:], xt[f][:, cs : cs + csz], wtv[f][:],
                        start=(f == 0), stop=(f == NF - 1),
                    )
            for hx in range(3):
                sb = kT_pool.tile([128, EXT], BF16, tag="kT")
                for j in range(2):
                    nc.vector.tensor_scalar_add(
                        sb[:, j * half : (j + 1) * half], kslot[(hx, j)][:],
                        bqk_t[H + hx][:],
                    )
                kT[hx] = sb
            for c in (0, 1):
                vsb = v_pool.tile([TCH[c][1], 512], BF16, tag="v")
                nc.scalar.activation(vsb[:], vps[c][:], COPY)
                vv[c][0] = vsb

        def emit_attention(h, fill_steps=None):
            si = 0
            sb = at_pool.tile([128, Q], BF16, tag=f"at{h}")
            for t in range(2):
                avp = av_ps.tile([128, 256], F32)
                dnp = dn_ps.tile([128, 256], F32)
                nch = len(CHUNKS[t])
                for ci, (cs, csz) in enumerate(CHUNKS[t]):
                    ptp = pt_ps.tile([csz, 256], F32)
                    nc.tensor.matmul(
                        ptp[:], kT[h][:, cs : cs + csz],
                        qT[h][:, t * 256 : (t + 1) * 256],
                        start=True, stop=True,
                    )
                    et = et_pool.tile([csz, 256], BF16, tag="et")
                    nc.scalar.activation(et[:], ptp[:], EXP)
                    ptm = ptm_pool.tile([csz, 256], BF16, tag="ptm")
                    nc.vector.tensor_mul(ptm[:], et[:], mask_t[t * 3 + ci][:csz, :])
                    nc.tensor.matmul(
                        avp[:], vv[cs // 128][h // 4][:csz, (h % 4) * 128 : (h % 4 + 1) * 128],
                        ptm[:], start=(ci == 0), stop=(ci == nch - 1),
                    )
                    nc.tensor.matmul(
                        dnp[:], ones_t[:csz, :], ptm[:],
                        start=(ci == 0), stop=(ci == nch - 1),
                    )
                    if fill_steps and si < len(fill_steps):
                        fill_steps[si]()
                        si += 1
                rec = rec_pool.tile([128, 256], F32, tag="rec")
                nc.vector.reciprocal_approx_fast(rec[:], dnp[:])
                nc.vector.tensor_mul(sb[:, t * 256 : (t + 1) * 256], avp[:], rec[:])
            at[h] = sb

        def q_fill_steps(hcur, wq_col):
            # Q(h+1) as six 2-matmul steps interleaved into attention(h)'s
            # chunks, filling the exp->mask-multiply pipeline stalls
            hx = hcur % 4
            ps = qk_ps.tile([128, Q], F32, name="ps")
            steps = []
            for fs in (range(0, 2), range(2, 4), range(4, 6),
                       range(6, 8), range(8, 10), range(10, 12)):
                def go(fs=fs, hcur=hcur, hx=hx):
                    for f in fs:
                        nc.tensor.matmul(
                            ps[:], wq_col(f, hx), xt[f][:, QCOL0 : QCOL0 + Q],
                            start=(f == 0), stop=(f == NF - 1),
                        )
                    if fs.stop == NF:
                        sb = qT_pool.tile([128, Q], BF16, tag="qT")
                        nc.scalar.activation(
                            sb[:], ps[:], IDENT, bias=bqk_t[hcur][:]
                        )
                        qT[hcur] = sb
                steps.append(go)
            return steps

        def k_hx0_fill(blkn, wt):
            # next block's first K head, split into 6 four-matmul steps that
            # fill the last attention head's pipeline stalls (PE is strict
            # FIFO: fill work must be emitted between the stalling matmuls)
            half = EXT // 2
            kfp = [qk_ps.tile([128, half], F32, name="ps") for _ in range(2)]
            sb = kT_pool.tile([128, EXT], BF16, tag="kT")
            steps = []
            for j in range(2):
                for fs in (range(0, 4), range(4, 8), range(8, 12)):
                    def go(j=j, fs=fs):
                        for f in fs:
                            nc.tensor.matmul(
                                kfp[j][:], wt[f][:, 0:128],
                                xt[f][:, j * half : (j + 1) * half],
                                start=(f == 0), stop=(f == NF - 1),
                            )
                        if fs.stop == NF:
                            nc.vector.tensor_scalar_add(
                                sb[:, j * half : (j + 1) * half], kfp[j][:],
                                bqk_t[H + blkn * 4][:],
                            )
                    steps.append(go)
            kT[blkn * 4] = sb
            return steps

        # ---- emission order per block: K (pipelines with incoming DMA
        # slices), V, then Q/attention with Q(h+1) ahead of attention(h).
        # blk0 starts with the joint K+V ramp loop.
        for blk in range(3):
            h0 = blk * 4
            if blk == 0:
                emit_kv_ramp(wt_g[3], wt_g[6])
                emit_k_block(blk, wt_g[3], hx0=3)
                emit_v_group(blk, wt_g[6], pairs=((2, 3), (4,)))
            else:
                emit_k_block(blk, wt_g[3 + blk], hx0=1)
                emit_v_group(blk, wt_g[6 + blk])
            wtb = wt_g[blk]
            wq_col = lambda f, hx, _w=wtb: _w[f][:, hx * 128 : (hx + 1) * 128]
            emit_q_chunk(h0, wq_col)
            for h in range(h0, h0 + 4):
                if h + 1 < h0 + 4:
                    fill = q_fill_steps(h + 1, wq_col)
                elif blk < 2:
                    fill = k_hx0_fill(blk + 1, wt_g[4 + blk])
                else:
                    fill = None
                emit_attention(h, fill)

        # release stage-1 PSUM pools (reverse alloc order) before the projection
        for p in (dn_ps, av_ps, pt_ps, v_ps, qk_ps):
            p.release()

        yt_sb_pool = tc.alloc_tile_pool(name="yt_sb", bufs=2)
        yt_ps = tc.alloc_tile_pool(name="yt_ps", bufs=2, space="PSUM")

        for og in range(3):
            wt = wt_o[og]
            for oc in range(4):
                o = og * 4 + oc
                if o < H - 1:
                    ps = yt_ps.tile([128, Q], F32)
                    for f in range(NF):
                        nc.tensor.matmul(
                            ps[:], wt[f][:, oc * 128 : (oc + 1) * 128], at[f][:],
                            start=(f == 0), stop=(f == NF - 1),
                        )
                    sb = yt_sb_pool.tile([128, Q], BF16, tag="yt")
                    nc.vector.tensor_scalar_add(sb[:], ps[:], bo_t[o][:])
                    nc.sync.dma_start(yt_d[o * 128 : (o + 1) * 128, :], sb[:])
                else:
                    # final chunk: two independent half-width chains (separate
                    # PSUM tiles, so no WAR serialization) — the first half's
                    # evac + store overlap the second half's matmul tail
                    for hf in range(2):
                        cols = slice(hf * 256, (hf + 1) * 256)
                        ps = yt_ps.tile([128, 256], F32, name="ps_h", tag="yt_h")
                        for f in range(NF):
                            nc.tensor.matmul(
                                ps[:], wt[f][:, oc * 128 : (oc + 1) * 128],
                                at[f][:, cols],
                                start=(f == 0), stop=(f == NF - 1),
                            )
                        sb = yt_sb_pool.tile([128, 256], BF16, tag="yt_h")
                        nc.vector.tensor_scalar_add(sb[:], ps[:], bo_t[o][:])
                        nc.sync.dma_start(
                            yt_d[o * 128 : (o + 1) * 128, cols], sb[:]
                        )

        yt_ps.release()
        for p in (yt_sb_pool, rec_pool, ptm_pool, et_pool, v_pool, kT_pool,
                  qT_pool, wq_pool, xt_pool, at_pool, const):
            p.release()

    nc.compile()
    return nc


def _softmax(v):
    e = np.exp(v - v.max())
    return e / e.sum()


def _build_masks(r0, gw):
    """Per-core mask tiles [6, 128, 256] routing softmax group weights onto the
    anchor positions of the transposed score chunks."""
    lo = r0 - WIN
    masks = np.zeros((6, 128, 256), np.float32)
    wts = [gw[0]] * 6 + [gw[1]] * 4
    for qi in range(Q):
        t, qq = divmod(qi, 256)

        def add(col, w):
            for ci, (cs, csz) in enumerate(CHUNKS[t]):
                if cs <= col < cs + csz:
                    masks[t * 3 + ci, col - cs, qq] += w
                    return
            raise AssertionError(f"col {col} not covered for qtile {t}")

        for off, w in zip(OFFS, wts):
            tok = min(max(r0 + qi + off, 0), S - 1)
            add(2 + (tok - lo), w)
        # global anchors: duplicated at both ends of the ext axis
        add(0 if t == 0 else EXT - 2, gw[2])   # token 0
        add(1 if t == 0 else EXT - 1, gw[2])   # token S-1
    return masks


def _prepare_in_maps(x, wqkv, bqkv, wout, bout, group_scale):
    scale = HD ** -0.5
    wqkv_m = np.array(wqkv, np.float32, copy=True)
    wqkv_m[:, :DIM] *= scale
    # pre-tile: [9 groups, 6 f-pairs, 128, 1024]; f-chunk f of group g lands
    # at flattened cols f*512 of the [128, NF*512] SBUF tile
    wqkv_t = np.ascontiguousarray(
        wqkv_m.reshape(NJ, 2, 128, 9, 512).transpose(3, 0, 2, 1, 4)
        .reshape(9, NJ, 128, 1024)
    ).astype(NPBF16)
    bqkv_m = np.array(bqkv, np.float32, copy=True)
    bqkv_m[:DIM] *= scale
    gw = _softmax(np.asarray(group_scale, np.float64))

    # V bias commutes through attention (softmax weights sum to 1): fold into
    # the output-projection bias
    wout_f = np.asarray(wout, np.float32)
    bout_eff = np.asarray(bout, np.float32) + bqkv_m[2 * DIM :] @ wout_f

    # bias columns [128, 36]: q heads, k heads, then out-proj chunks
    bcol = np.concatenate(
        [
            bqkv_m[:DIM].reshape(H, 128),
            bqkv_m[DIM : 2 * DIM].reshape(H, 128),
            bout_eff.reshape(H, 128),
        ],
        axis=0,
    ).T.astype(np.float32).copy()  # [128, 36]
    wout_t = np.ascontiguousarray(
        wout_f.reshape(NJ, 2, 128, 3, 512)
        .transpose(3, 0, 2, 1, 4).reshape(3, NJ, 128, 1024)
    ).astype(NPBF16)
    ones_sq = np.ones((128, 128), NPBF16)

    in_maps = []
    for core in range(NCORES):
        b, sc = divmod(core, SCHUNKS)
        r0 = sc * Q
        lo = r0 - WIN
        tok_ids = np.concatenate(
            [
                [0, S - 1],
                np.clip(np.arange(lo, lo + Q + 2 * WIN), 0, S - 1),
                [0, S - 1],
            ]
        ).astype(np.int64)
        x_ext_t = np.ascontiguousarray(x[b, tok_ids, :].T)  # [DIM, EXT]
        # pack f-chunk pairs: [NJ, 128, 2*EXT]
        xt_p = np.ascontiguousarray(
            x_ext_t.reshape(NJ, 2, 128, EXT).transpose(0, 2, 1, 3)
            .reshape(NJ, 128, 2 * EXT)
        ).astype(NPBF16)
        masks = _build_masks(r0, gw).astype(NPBF16)
        in_maps.append(
            {
                "xt": xt_p,
                "wqkv": wqkv_t,
                "wout": wout_t,
                "bcol": bcol,
                "ones_sq": ones_sq,
                "masks": masks,
            }
        )
    return in_maps


def get_program():
    if "nc" not in _CACHE:
        _CACHE["nc"] = _build_program()
    return _CACHE["nc"]


def run(inputs, **spmd_kwargs):
    """Run the SPMD kernel; returns (y [B,S,DIM] fp32, BassKernelResults)."""
    x = np.asarray(inputs["x"], np.float32)
    in_maps = _prepare_in_maps(
        x,
        np.asarray(inputs["Wqkv"], np.float32),
        np.asarray(inputs["bqkv"], np.float32),
        np.asarray(inputs["Wout"], np.float32),
        np.asarray(inputs["bout"], np.float32),
        np.asarray(inputs["group_scale"], np.float32),
    )
    nc = get_program()
    res = bass_utils.run_bass_kernel_spmd(
        nc, in_maps, core_ids=list(range(NCORES)), **spmd_kwargs
    )
    y = np.empty((B, S, DIM), np.float32)
    for core in range(NCORES):
        b, sc = divmod(core, SCHUNKS)
        y[b, sc * Q : (sc + 1) * Q, :] = res.results[core]["yt"].astype(np.float32).T
    return y, res


def kernel(**inputs):
    y, _ = run(inputs)
    return y
